# revision 50
# baseline (speedup 1.0000x reference)
"""Trainium2 Bass kernel for nn_BoundaryLoss: boundary-weighted softmax MSE.

Fully local (no collectives), 8 NeuronCores:
  core c: b = c//4, D-slab of 24 planes starting d0 = 24*(c%4), extended by
  an S-plane halo per side (E = 24+2S planes).

  The host ships the W-pass of the separable squared-EDT directly: the 1-D
  distance to the nearest boundary voxel along W is already computed on the
  host to choose the window S, and for the binary boundary seed the W-pass
  output is exactly that distance squared (BIG for lines with no boundary;
  out-of-volume halo planes BIG).

  Device EDT in L1 = (96 h-partitions, free = (E d-planes x 96 w)):
    pass D: plane-strided shifts (3 groups of 8 planes). Per group:
    PE-transpose -> L2 (96 w-parts, free (8 x padded-h)) -> pass H (DVE)
    -> PE-transpose back with ACT evac fusing y = sqrt(d2)/theta ->
    w_g = exp(-y_g) (accum_out gives sum(w_g) free) -> per-group tail.

  Loss via sum_c (p_c - t_c)^2 = S2*r^2 - 2*e_t*r + 1:
    e_c = exp(pred_c)        (ACT, class chunks)
    Z = sum_c e_c            (DVE pair-adds), lnZ = Ln(Z), r = exp(-lnZ)
    e2 = e*e (DVE), per-group S2 = sum_c e2_c (DVE pair-adds)
    t4 = r*(S2*r - e2t)      (DVE; e2t = exp(pred_t + ln2), host-gathered)
    t6 = t4*w_g (DVE), ACT Copy+accum -> per-partition partials in accT
    loss = sum(accT over cores) / n_vox   (host sums the 8x96x6 partials)

Exactness: S = max over W-lines of the 1-D W-distance (exact host scans),
so the shipped seed fw = dist_w^2 <= S^2 pointwise; the D and H passes
operate on fields bounded by S^2, so any of their minimizers lies within
S. Squared distances are small integers (<= 3*S^2), exact in bf16 up to
256. S is capped at 10 (SBUF); inputs that would need more (near-empty
boundary sets) only differ where exp(-dist/theta) underflows.

Input envelope: softmax is computed without max-subtraction (spec'd pred is
randn, so exp stays in [e-6, e6]); logits beyond ~23 would overflow the
hardware exp table via exp(2x). pred is shipped bf16 (rel-err ~0.4% per
voxel, unbiased, averaged over 1.7M voxels; tolerance is 2e-2).
"""
import sys

sys.path.insert(0, "/opt/trn_rl_repo")

import math

import numpy as np
import ml_dtypes

import concourse.bass as bass
import concourse.mybir as mybir
import concourse.tile as tile
from concourse import masks
from concourse.bass_utils import run_bass_kernel_spmd

AF = mybir.ActivationFunctionType
ALU = mybir.AluOpType
BF16 = mybir.dt.bfloat16
F32 = mybir.dt.float32

_MAXW = 1  # walrus CoreV3 in this toolchain rejects >1 sync wait per instruction


def _split_multi_waits(nc):
    """Split instructions carrying multiple sem waits into NoOp prefixes.

    The Tile tail-drain waits on every used semaphore lane in one Drain;
    this walrus build only codegens a single sync-wait command per
    instruction, so move extra waits onto preceding same-engine NoOps."""
    for fn in nc.m.functions:
        for bb in fn.blocks:
            insts = list(bb.instructions)
            out = []
            for ins in insts:
                si = ins.sync_info
                if si is not None and si.on_wait is not None and len(si.on_wait) > _MAXW:
                    waits = list(si.on_wait)
                    extra, keep = waits[:-_MAXW], waits[-_MAXW:]
                    while extra:
                        chunk, extra = extra[:_MAXW], extra[_MAXW:]
                        out.append(mybir.InstNoOp(
                            name=nc.get_next_instruction_name(),
                            engine=ins.engine,
                            sync_info=mybir.SyncInfo(on_wait=chunk, on_update=[]),
                            bass_nofuse=True,
                        ))
                    si.on_wait = keep
                out.append(ins)
            bb.instructions = out
    return nc


B, C, D, H, W = 2, 4, 96, 96, 96
N_CORES = 8
DS = D // 4          # 24: per-core D-slab
G = 8                # d-plane group size for pipelining (DS = 3*G)
NG = DS // G
THETA = 5.0
BIG = 1e10
LN2 = math.log(2.0)

# tuning knobs (validated by timeline sim)
H_ON_GP = (False,) * 8   # per-group: H-pass on GPSIMD vs DVE (GP TT illegal on HW)
N_E2_ACT = 0                    # classes of e2 via ACT exp(2x); rest DVE e*e
R2_ON_ACT = True
EVAC_ON_GP = False
PER_GROUP_E2 = False
LADDER_HALVES = False
LAST_RED_DVE = True
PRED_DMA_CH = 2
N_E_CHUNKS = 8
E2_BY_GROUP = False
WR_FOLD = True
SPLIT_LAST_TAIL = False  # splitting loses: op overheads > chain gain               # r2 = exp(-2 lnZ) on ACT vs r*r on DVE
# interleaved emission order for h-groups and bulk loss ACT ops
EMIT_ORDER = [("e", i) for i in range(8)] + [("e2t", 0), ("zp", 0)] + \
    [("h", 0), ("h", 1), ("h", 2)] + \
    [("e2", 0), ("e2", 1), ("e2", 2), ("e2", 3)]


def _wline_dist(target: np.ndarray) -> np.ndarray:
    """Exact 1-D distance to the nearest boundary voxel along W (per line).
    INF (1<<20) where a line has no boundary voxel."""
    bnd = _boundary(target)
    INF = 1 << 20
    dist = np.where(bnd, 0, INF)
    for i in range(1, W):
        np.minimum(dist[..., i], dist[..., i - 1] + 1, out=dist[..., i])
    for i in range(W - 2, -1, -1):
        np.minimum(dist[..., i], dist[..., i + 1] + 1, out=dist[..., i])
    return dist


def _required_window(dist: np.ndarray) -> int:
    """Smallest window S such that the windowed min-conv (D, H pass order)
    on the host-shipped W-pass seed equals the full min-conv.

    S = max over W-lines of the 1-D distance to the nearest boundary voxel
    along W. The seed fw = dist^2 is bounded by S^2 pointwise, so any D/H
    minimizer is within S. 95 (-> the 10 cap) if some line is empty."""
    m = int(dist.max())
    return 95 if m >= (1 << 20) else m


def _window_for(dist: np.ndarray) -> int:
    return min(max(_required_window(dist), 2), 10)


def _boundary(target: np.ndarray) -> np.ndarray:
    gd = target[:, 1:, :, :] != target[:, :-1, :, :]
    gh = target[:, :, 1:, :] != target[:, :, :-1, :]
    gw = target[:, :, :, 1:] != target[:, :, :, :-1]
    bnd = np.zeros(target.shape, np.bool_)
    bnd[:, :-1] |= gd
    bnd[:, :, :-1] |= gh
    bnd[:, :, :, :-1] |= gw
    return bnd


def _edt_range(eng, pool, fsrc, out, a, b, S, tag):
    """Windowed squared-EDT min-conv along the free axis on cols [a, b).

    fsrc/out: (96, FD) fields of padded lines (pads BIG); [a, b) must start
    and end at plane boundaries so the unwritten out cols [a,a+s)/[b-s,b)
    are pads. out[c] = min_{|s|<=S} fsrc[c+s] + s^2 on all real columns."""
    n = b - a
    for s in range(1, S + 1):
        u = pool.tile([96, n - 2 * s], BF16, name=f"u_{tag}_{s}")
        eng.tensor_tensor(
            u[:, :], fsrc[:, a : b - 2 * s], fsrc[:, a + 2 * s : b], ALU.min
        )
        eng.tensor_scalar(u[:, :], u[:, :], float(s * s), None, ALU.add)
        if s == 1:
            # first shift also plays the s=0 init: out = min(fsrc, u1+1)
            eng.tensor_tensor(
                out[:, a + s : b - s], fsrc[:, a + s : b - s], u[:, :], ALU.min
            )
        else:
            eng.tensor_tensor(
                out[:, a + s : b - s], out[:, a + s : b - s], u[:, :], ALU.min
            )


def build_nc(S: int) -> bass.Bass:
    E = DS + 2 * S        # extended slab planes (with halo)
    PAD = S + (S % 2)     # even in-line pad: keeps bf16 APs 4B-aligned
    Lh = 96 + 2 * PAD     # padded h-line length
    CW = DS * 96          # per-partition voxels (2304)
    GW = G * 96           # per-group voxels (768)

    nc = bass.Bass(num_devices=N_CORES)

    seed_in = nc.dram_tensor("seed", [H, E * 96], BF16, kind="ExternalInput")
    pred_in = nc.dram_tensor("predh", [H, C * DS * W], BF16, kind="ExternalInput")
    pt2_in = nc.dram_tensor("predt2", [H, DS * W], BF16, kind="ExternalInput")
    out_part = nc.dram_tensor("partial", [96, 2 * NG], F32, kind="ExternalOutput")

    with tile.TileContext(nc) as tc:
        with (
            tc.tile_pool(name="pool", bufs=1) as pool,
            tc.tile_pool(name="psum", bufs=1, space="PSUM") as psum,
        ):
            ident = pool.tile([128, 128], BF16)
            masks.make_identity(nc, ident[:])

            # ---- input DMAs, critical-first; seed is the host-computed
            # W-pass output fw = (1-D W-line distance)^2, halo planes BIG
            fw = pool.tile([96, E, 96], BF16, name="fw")
            fwf = fw.rearrange("p a b -> p (a b)")
            SEED0 = (S + G + S) * 96   # planes D-group-0 reads
            nc.sync.dma_start(fwf[:, :SEED0], seed_in[:, :SEED0])
            nc.sync.dma_start(fwf[:, SEED0:], seed_in[:, SEED0:])
            P_ = pool.tile([96, C, CW], BF16, name="P_")
            Pf = P_.rearrange("h c f -> h (c f)")
            for k in range(PRED_DMA_CH):
                a0 = k * C * CW // PRED_DMA_CH
                a1 = (k + 1) * C * CW // PRED_DMA_CH
                nc.sync.dma_start(Pf[:, a0:a1], pred_in[:, a0:a1])
            pt2 = pool.tile([96, CW], BF16, name="pt2")
            nc.sync.dma_start(pt2[:, :], pt2_in[:, :])

            # ---- f2 pads (off-chain, GP)
            f2 = pool.tile([96, DS, Lh], BF16, name="f2")
            nc.gpsimd.memset(f2[:, :, 0:PAD], BIG)
            nc.gpsimd.memset(f2[:, :, PAD + 96 : Lh], BIG)
            f2f = f2.rearrange("p a b -> p (a b)")
            fh = pool.tile([96, DS, Lh], BF16, name="fh")
            fhf = fh.rearrange("p a b -> p (a b)")

            fwv = fw
            y = pool.tile([96, DS, 96], BF16, name="y")
            wgt = pool.tile([96, CW], BF16, name="wgt")
            junk = pool.tile([96, CW], BF16, name="junk")
            t4 = pool.tile([96, CW], BF16, name="t4")
            accT = pool.tile([96, 2 * NG], F32, name="accT")

            def emit_d_group(g):
                g0 = g * G
                fd = pool.tile([96, G, 96], BF16, name=f"fd_{g}")
                for s in range(1, S + 1):
                    ud = pool.tile([96, G, 96], BF16, name=f"ud_{g}_{s}")
                    nc.vector.tensor_tensor(
                        ud[:],
                        fwv[:, S + g0 - s : S + g0 + G - s, :],
                        fwv[:, S + g0 + s : S + g0 + G + s, :],
                        ALU.min,
                    )
                    nc.vector.tensor_scalar(ud[:], ud[:], float(s * s), None,
                                            ALU.add)
                    if s == 1:
                        nc.vector.tensor_tensor(
                            fd[:], fwv[:, S + g0 : S + g0 + G, :],
                            ud[:], ALU.min,
                        )
                    else:
                        nc.vector.tensor_tensor(fd[:], fd[:], ud[:], ALU.min)
                # transpose group planes into L2; ACT evacuates PSUM
                pt = psum.tile([96, GW], BF16, name=f"pt_{g}", tag="pt",
                               bufs=2)
                for k in range(G):
                    nc.tensor.transpose(pt[:, k * 96 : (k + 1) * 96],
                                        fd[:, k, :], ident[:96, :96])
                if EVAC_ON_GP:
                    # window-1 avg-pool == copy; runs on the idle GPSIMD
                    nc.gpsimd.pool(
                        f2[:, g0 : g0 + G, PAD : PAD + 96],
                        pt[:, :].rearrange("p (k w) -> p (k w) 1"),
                        mybir.PoolFunctionType.avg,
                    )
                else:
                    nc.scalar.activation(
                        f2[:, g0 : g0 + G, PAD : PAD + 96],
                        pt[:, :].rearrange("p (k w) -> p k w", k=G),
                        AF.Copy,
                    )

            def emit_h_body(g):
                g0 = g * G
                eng = nc.gpsimd if H_ON_GP[g] else nc.vector
                _edt_range(eng, pool, f2f, fhf, g0 * Lh, (g0 + G) * Lh, S,
                           f"h{g}")
                # transpose back into PSUM (evac'd later by the sqrt)
                ptb = psum.tile([96, GW], BF16, name=f"ptb_{g}", tag="pt",
                                bufs=2)
                for k in range(G):
                    nc.tensor.transpose(
                        ptb[:, k * 96 : (k + 1) * 96],
                        fh[:, g0 + k, PAD : PAD + 96], ident[:96, :96],
                    )
                return ptb

            def emit_h_tail(g, ptb):
                # evac fuses y = sqrt(d2)/theta; w = exp(-y) with free sum(w)
                g0 = g * G
                nc.scalar.activation(
                    y[:, g0 : g0 + G, :],
                    ptb[:, :].rearrange("p (k w) -> p k w", k=G),
                    AF.Sqrt, scale=1.0 / (THETA * THETA),
                )
                nc.scalar.activation(
                    wgt[:, g * GW : (g + 1) * GW],
                    y[:, g0 : g0 + G, :].rearrange("p a b -> p (a b)"),
                    AF.Exp, scale=-1.0, accum_out=accT[:, g : g + 1],
                )

            # ---- EDT emission: D groups (W-pass shipped from host)
            emit_d_group(0)
            emit_d_group(1)
            emit_d_group(2)
            # ---- loss bulk ACT work (emitted per EMIT_ORDER interleave)
            NE = N_E_CHUNKS  # e chunks (fine so ACT can yield to evacs)
            e = pool.tile([96, C, CW], BF16, name="e")
            ef = e.rearrange("h c f -> h (c f)")
            EC = C * CW // NE

            def emit_e(i):
                nc.scalar.activation(ef[:, i * EC : (i + 1) * EC],
                                     Pf[:, i * EC : (i + 1) * EC], AF.Exp)

            e2t = pool.tile([96, CW], BF16, name="e2t")
            e2 = pool.tile([96, C, CW], BF16, name="e2")

            def emit_e2(c):
                if c < N_E2_ACT:
                    nc.scalar.activation(e2[:, c, :], P_[:, c, :], AF.Exp,
                                         scale=2.0)
                elif E2_BY_GROUP:
                    # c encodes (group, class): finer chunks unblock the
                    # per-group sp consumers earlier
                    gg, cc = divmod(c, C)
                    sl = slice(gg * GW, (gg + 1) * GW)
                    nc.vector.tensor_tensor(e2[:, cc, sl], e[:, cc, sl],
                                            e[:, cc, sl], ALU.mult)
                else:
                    nc.vector.tensor_tensor(e2[:, c, :], e[:, c, :],
                                            e[:, c, :], ALU.mult)

            zp = pool.tile([96, 2, CW], BF16, name="zp")
            Z = pool.tile([96, CW], BF16, name="Z")
            sp = pool.tile([96, 2, CW], BF16, name="sp")
            S2 = pool.tile([96, CW], BF16, name="S2")
            lnZ = pool.tile([96, CW], F32, name="lnZ")
            r = pool.tile([96, CW], BF16, name="r")
            ptbs = [None] * NG
            for item in EMIT_ORDER:
                kind, idx = item
                if kind == "h":
                    ptbs[idx] = emit_h_body(idx)
                elif kind == "e":
                    emit_e(idx)
                elif kind == "e2":
                    emit_e2(idx)
                elif kind == "e2t":
                    nc.scalar.activation(e2t[:, :], pt2[:, :], AF.Exp)
                elif kind == "zp":
                    if LADDER_HALVES:
                        for hh in range(2):
                            sl = slice(hh * CW // 2, (hh + 1) * CW // 2)
                            nc.vector.tensor_tensor(
                                zp[:, :, sl], e[:, 0:2, sl], e[:, 2:4, sl],
                                ALU.add)
                            nc.vector.tensor_tensor(
                                Z[:, sl], zp[:, 0, sl], zp[:, 1, sl], ALU.add)
                            nc.scalar.activation(lnZ[:, sl], Z[:, sl], AF.Ln)
                            nc.scalar.activation(r[:, sl], lnZ[:, sl], AF.Exp,
                                                 scale=-1.0)
                    else:
                        nc.vector.tensor_tensor(zp[:], e[:, 0:2, :],
                                                e[:, 2:4, :], ALU.add)
                        nc.vector.tensor_tensor(Z[:], zp[:, 0, :],
                                                zp[:, 1, :], ALU.add)

            # ---- softmax chain (Z/S2 pair-adds emitted via EMIT_ORDER)
            if not LADDER_HALVES:
                nc.scalar.activation(lnZ[:, :], Z[:, :], AF.Ln)
                nc.scalar.activation(r[:, :], lnZ[:, :], AF.Exp, scale=-1.0)
            # per-group tail: t4 = r*(S2*r - e2t) factored (no r2 op);
            # group chains interleave with the H-pass groups on DVE
            m1 = pool.tile([96, CW], BF16, name="m1")

            def emit_tail_slice(g, sl, hh):
                nc.vector.tensor_tensor(sp[:, :, sl], e2[:, 0:2, sl],
                                        e2[:, 2:4, sl], ALU.add)
                nc.vector.tensor_tensor(S2[:, sl], sp[:, 0, sl],
                                        sp[:, 1, sl], ALU.add)
                if WR_FOLD:
                    nc.vector.tensor_tensor(t4[:, sl], wgt[:, sl], r[:, sl],
                                            ALU.mult)
                nc.vector.tensor_tensor(m1[:, sl], S2[:, sl], r[:, sl],
                                        ALU.mult)
                nc.vector.tensor_tensor(m1[:, sl], m1[:, sl], e2t[:, sl],
                                        ALU.subtract)
                if WR_FOLD:
                    nc.vector.tensor_tensor(junk[:, sl], m1[:, sl], t4[:, sl],
                                            ALU.mult)
                else:
                    nc.vector.tensor_tensor(t4[:, sl], m1[:, sl], r[:, sl],
                                            ALU.mult)
                    nc.vector.tensor_tensor(junk[:, sl], t4[:, sl], wgt[:, sl],
                                            ALU.mult)
                if LAST_RED_DVE and g == NG - 1:
                    nc.vector.tensor_reduce(
                        accT[:, NG + g + hh : NG + g + hh + 1],
                        junk[:, sl], op=ALU.add,
                        axis=mybir.AxisListType.X,
                    )
                else:
                    nc.scalar.activation(
                        y[:, g * G : (g + 1) * G, :],
                        junk[:, sl].rearrange("p (a b) -> p a b", b=96),
                        AF.Copy, accum_out=accT[:, NG + g : NG + g + 1],
                    )

            for g in range(NG):
                emit_h_tail(g, ptbs[g])
                nhalf = 2 if (SPLIT_LAST_TAIL and g == NG - 1) else 1
                for hh in range(nhalf):
                    a0 = g * GW + hh * GW // nhalf
                    emit_tail_slice(g, slice(a0, a0 + GW // nhalf), hh)

            nc.sync.dma_start(out_part[:, :], accT[:, :])

    _split_multi_waits(nc)
    return nc


_cache: dict[int, bass.Bass] = {}


def make_in_maps(pred: np.ndarray, target: np.ndarray, S: int,
                 dist: np.ndarray) -> list:
    E = DS + 2 * S
    # W-pass output: squared 1-D W-line distance (<= S^2, exact in bf16);
    # BIG where the line has no boundary voxel
    seed_full = np.where(
        dist < (1 << 20), (dist.astype(np.int64) ** 2).astype(np.float64), BIG
    ).astype(ml_dtypes.bfloat16)                                     # (B,D,H,W)
    pred_bf = pred.astype(ml_dtypes.bfloat16)
    # host gather of the target-class logit, with ln2 folded in
    pt2_full = (
        np.take_along_axis(pred, target[:, None], axis=1)[:, 0] + LN2
    ).astype(ml_dtypes.bfloat16)                                     # (B,D,H,W)
    in_maps = []
    for core in range(N_CORES):
        b, i = divmod(core, 4)
        d0 = i * DS
        dg = np.arange(d0 - S, d0 + DS + S)          # global plane ids, may be OOR
        inr = (dg >= 0) & (dg < D)
        seed = np.full((E, H, 96), BIG, ml_dtypes.bfloat16)
        seed[inr] = seed_full[b][dg[inr]]
        in_maps.append({
            "seed": np.ascontiguousarray(
                seed.transpose(1, 0, 2).reshape(H, E * 96)
            ),
            "predh": np.ascontiguousarray(
                pred_bf[b, :, d0 : d0 + DS].transpose(2, 0, 1, 3)
            ).reshape(H, C * DS * W),
            "predt2": np.ascontiguousarray(
                pt2_full[b, d0 : d0 + DS].transpose(1, 0, 2)
            ).reshape(H, DS * W),
        })
    return in_maps


def kernel(pred: np.ndarray, target: np.ndarray) -> np.ndarray:
    pred = np.ascontiguousarray(pred, np.float32)
    target = np.ascontiguousarray(target, np.int32)
    dist = _wline_dist(target)
    S = _window_for(dist)

    if S not in _cache:
        _cache[S] = build_nc(S)
    nc = _cache[S]

    in_maps = make_in_maps(pred, target, S, dist)
    res = run_bass_kernel_spmd(nc, in_maps, core_ids=list(range(N_CORES)))
    total = sum(float(r["partial"].sum()) for r in res.results)
    n_vox = float(B * D * H * W)
    return np.array(total / n_vox, dtype=np.float32)


# revision 51
# speedup vs baseline: 1.0149x; 1.0149x over previous
"""Trainium2 Bass kernel for nn_BoundaryLoss: boundary-weighted softmax MSE.

Fully local (no collectives), 8 NeuronCores:
  core c: b = c//4, D-slab of 24 planes starting d0 = 24*(c%4), extended by
  an S-plane halo per side (E = 24+2S planes).

  The host ships the W-pass of the separable squared-EDT directly: the 1-D
  distance to the nearest boundary voxel along W is already computed on the
  host to choose the window S, and for the binary boundary seed the W-pass
  output is exactly that distance squared (BIG for lines with no boundary;
  out-of-volume halo planes BIG).

  Device EDT in L1 = (96 h-partitions, free = (E d-planes x 96 w)):
    pass D: plane-strided shifts (3 groups of 8 planes). Per group:
    PE-transpose -> L2 (96 w-parts, free (8 x padded-h)) -> pass H (DVE)
    -> PE-transpose back with ACT evac fusing y = sqrt(d2)/theta ->
    w_g = exp(-y_g) (accum_out gives sum(w_g) free) -> per-group tail.

  Loss via sum_c (p_c - t_c)^2 = S2*r^2 - 2*e_t*r + 1:
    e_c = exp(pred_c)        (ACT, class chunks)
    Z = sum_c e_c            (DVE pair-adds), lnZ = Ln(Z), r = exp(-lnZ)
    e2 = e*e (DVE), per-group S2 = sum_c e2_c (DVE pair-adds)
    t4 = r*(S2*r - e2t)      (DVE; e2t = exp(pred_t + ln2), host-gathered)
    t6 = t4*w_g (DVE), ACT Copy+accum -> per-partition partials in accT
    loss = sum(accT over cores) / n_vox   (host sums the 8x96x6 partials)

Exactness: S = max over W-lines of the 1-D W-distance (exact host scans),
so the shipped seed fw = dist_w^2 <= S^2 pointwise; the D and H passes
operate on fields bounded by S^2, so any of their minimizers lies within
S. Squared distances are small integers (<= 3*S^2), exact in bf16 up to
256. S is capped at 10 (SBUF); inputs that would need more (near-empty
boundary sets) only differ where exp(-dist/theta) underflows.

Input envelope: softmax is computed without max-subtraction (spec'd pred is
randn, so exp stays in [e-6, e6]); logits beyond ~23 would overflow the
hardware exp table via exp(2x). pred is shipped bf16 (rel-err ~0.4% per
voxel, unbiased, averaged over 1.7M voxels; tolerance is 2e-2).
"""
import sys

sys.path.insert(0, "/opt/trn_rl_repo")

import math

import numpy as np
import ml_dtypes

import concourse.bass as bass
import concourse.mybir as mybir
import concourse.tile as tile
from concourse import masks
from concourse.bass_utils import run_bass_kernel_spmd

AF = mybir.ActivationFunctionType
ALU = mybir.AluOpType
BF16 = mybir.dt.bfloat16
F32 = mybir.dt.float32

_MAXW = 1  # walrus CoreV3 in this toolchain rejects >1 sync wait per instruction


def _split_multi_waits(nc):
    """Split instructions carrying multiple sem waits into NoOp prefixes.

    The Tile tail-drain waits on every used semaphore lane in one Drain;
    this walrus build only codegens a single sync-wait command per
    instruction, so move extra waits onto preceding same-engine NoOps."""
    for fn in nc.m.functions:
        for bb in fn.blocks:
            insts = list(bb.instructions)
            out = []
            for ins in insts:
                si = ins.sync_info
                if si is not None and si.on_wait is not None and len(si.on_wait) > _MAXW:
                    waits = list(si.on_wait)
                    extra, keep = waits[:-_MAXW], waits[-_MAXW:]
                    while extra:
                        chunk, extra = extra[:_MAXW], extra[_MAXW:]
                        out.append(mybir.InstNoOp(
                            name=nc.get_next_instruction_name(),
                            engine=ins.engine,
                            sync_info=mybir.SyncInfo(on_wait=chunk, on_update=[]),
                            bass_nofuse=True,
                        ))
                    si.on_wait = keep
                out.append(ins)
            bb.instructions = out
    return nc


B, C, D, H, W = 2, 4, 96, 96, 96
N_CORES = 8
DS = D // 4          # 24: per-core D-slab
G = 6                # d-plane group size for pipelining (DS = 4*G)
NG = DS // G
THETA = 5.0
BIG = 1e10
LN2 = math.log(2.0)

# tuning knobs (validated by timeline sim)
H_ON_GP = (False,) * 8   # per-group: H-pass on GPSIMD vs DVE (GP TT illegal on HW)
N_E2_ACT = 0                    # classes of e2 via ACT exp(2x); rest DVE e*e
R2_ON_ACT = True
EVAC_ON_GP = False
PER_GROUP_E2 = False
LADDER_HALVES = False
LAST_RED_DVE = True
PRED_DMA_CH = 2
N_E_CHUNKS = 8
E2_BY_GROUP = False
WR_FOLD = True
SPLIT_LAST_TAIL = False  # splitting loses: op overheads > chain gain               # r2 = exp(-2 lnZ) on ACT vs r*r on DVE
# interleaved emission order for h-groups and bulk loss ACT ops
EMIT_ORDER = [("e", i) for i in range(8)] + [("e2t", 0), ("zp", 0)] + \
    [("h", 0), ("h", 1), ("h", 2), ("h", 3)] + \
    [("e2", 0), ("e2", 1), ("e2", 2), ("e2", 3)]


def _wline_dist(target: np.ndarray) -> np.ndarray:
    """Exact 1-D distance to the nearest boundary voxel along W (per line).
    INF (1<<20) where a line has no boundary voxel."""
    bnd = _boundary(target)
    INF = 1 << 20
    dist = np.where(bnd, 0, INF)
    for i in range(1, W):
        np.minimum(dist[..., i], dist[..., i - 1] + 1, out=dist[..., i])
    for i in range(W - 2, -1, -1):
        np.minimum(dist[..., i], dist[..., i + 1] + 1, out=dist[..., i])
    return dist


def _required_window(dist: np.ndarray) -> int:
    """Smallest window S such that the windowed min-conv (D, H pass order)
    on the host-shipped W-pass seed equals the full min-conv.

    S = max over W-lines of the 1-D distance to the nearest boundary voxel
    along W. The seed fw = dist^2 is bounded by S^2 pointwise, so any D/H
    minimizer is within S. 95 (-> the 10 cap) if some line is empty."""
    m = int(dist.max())
    return 95 if m >= (1 << 20) else m


def _window_for(dist: np.ndarray) -> int:
    return min(max(_required_window(dist), 2), 10)


def _boundary(target: np.ndarray) -> np.ndarray:
    gd = target[:, 1:, :, :] != target[:, :-1, :, :]
    gh = target[:, :, 1:, :] != target[:, :, :-1, :]
    gw = target[:, :, :, 1:] != target[:, :, :, :-1]
    bnd = np.zeros(target.shape, np.bool_)
    bnd[:, :-1] |= gd
    bnd[:, :, :-1] |= gh
    bnd[:, :, :, :-1] |= gw
    return bnd


def _edt_range(eng, pool, fsrc, out, a, b, S, tag):
    """Windowed squared-EDT min-conv along the free axis on cols [a, b).

    fsrc/out: (96, FD) fields of padded lines (pads BIG); [a, b) must start
    and end at plane boundaries so the unwritten out cols [a,a+s)/[b-s,b)
    are pads. out[c] = min_{|s|<=S} fsrc[c+s] + s^2 on all real columns."""
    n = b - a
    for s in range(1, S + 1):
        u = pool.tile([96, n - 2 * s], BF16, name=f"u_{tag}_{s}")
        eng.tensor_tensor(
            u[:, :], fsrc[:, a : b - 2 * s], fsrc[:, a + 2 * s : b], ALU.min
        )
        eng.tensor_scalar(u[:, :], u[:, :], float(s * s), None, ALU.add)
        if s == 1:
            # first shift also plays the s=0 init: out = min(fsrc, u1+1)
            eng.tensor_tensor(
                out[:, a + s : b - s], fsrc[:, a + s : b - s], u[:, :], ALU.min
            )
        else:
            eng.tensor_tensor(
                out[:, a + s : b - s], out[:, a + s : b - s], u[:, :], ALU.min
            )


def build_nc(S: int) -> bass.Bass:
    E = DS + 2 * S        # extended slab planes (with halo)
    PAD = S + (S % 2)     # even in-line pad: keeps bf16 APs 4B-aligned
    Lh = 96 + 2 * PAD     # padded h-line length
    CW = DS * 96          # per-partition voxels (2304)
    GW = G * 96           # per-group voxels (768)

    nc = bass.Bass(num_devices=N_CORES)

    seed_in = nc.dram_tensor("seed", [H, E * 96], BF16, kind="ExternalInput")
    pred_in = nc.dram_tensor("predh", [H, C * DS * W], BF16, kind="ExternalInput")
    pt2_in = nc.dram_tensor("predt2", [H, DS * W], BF16, kind="ExternalInput")
    out_part = nc.dram_tensor("partial", [96, 2 * NG], F32, kind="ExternalOutput")

    with tile.TileContext(nc) as tc:
        with (
            tc.tile_pool(name="pool", bufs=1) as pool,
            tc.tile_pool(name="psum", bufs=1, space="PSUM") as psum,
        ):
            ident = pool.tile([128, 128], BF16)
            masks.make_identity(nc, ident[:])

            # ---- input DMAs, critical-first; seed is the host-computed
            # W-pass output fw = (1-D W-line distance)^2, halo planes BIG
            fw = pool.tile([96, E, 96], BF16, name="fw")
            fwf = fw.rearrange("p a b -> p (a b)")
            SEED0 = (S + G + S) * 96   # planes D-group-0 reads
            nc.sync.dma_start(fwf[:, :SEED0], seed_in[:, :SEED0])
            nc.sync.dma_start(fwf[:, SEED0:], seed_in[:, SEED0:])
            P_ = pool.tile([96, C, CW], BF16, name="P_")
            Pf = P_.rearrange("h c f -> h (c f)")
            for k in range(PRED_DMA_CH):
                a0 = k * C * CW // PRED_DMA_CH
                a1 = (k + 1) * C * CW // PRED_DMA_CH
                nc.sync.dma_start(Pf[:, a0:a1], pred_in[:, a0:a1])
            pt2 = pool.tile([96, CW], BF16, name="pt2")
            nc.sync.dma_start(pt2[:, :], pt2_in[:, :])

            # ---- f2 pads (off-chain, GP)
            f2 = pool.tile([96, DS, Lh], BF16, name="f2")
            nc.gpsimd.memset(f2[:, :, 0:PAD], BIG)
            nc.gpsimd.memset(f2[:, :, PAD + 96 : Lh], BIG)
            f2f = f2.rearrange("p a b -> p (a b)")
            fh = pool.tile([96, DS, Lh], BF16, name="fh")
            fhf = fh.rearrange("p a b -> p (a b)")

            fwv = fw
            y = pool.tile([96, DS, 96], BF16, name="y")
            wgt = pool.tile([96, CW], BF16, name="wgt")
            junk = pool.tile([96, CW], BF16, name="junk")
            t4 = pool.tile([96, CW], BF16, name="t4")
            accT = pool.tile([96, 2 * NG], F32, name="accT")

            def emit_d_group(g):
                g0 = g * G
                fd = pool.tile([96, G, 96], BF16, name=f"fd_{g}")
                for s in range(1, S + 1):
                    ud = pool.tile([96, G, 96], BF16, name=f"ud_{g}_{s}")
                    nc.vector.tensor_tensor(
                        ud[:],
                        fwv[:, S + g0 - s : S + g0 + G - s, :],
                        fwv[:, S + g0 + s : S + g0 + G + s, :],
                        ALU.min,
                    )
                    nc.vector.tensor_scalar(ud[:], ud[:], float(s * s), None,
                                            ALU.add)
                    if s == 1:
                        nc.vector.tensor_tensor(
                            fd[:], fwv[:, S + g0 : S + g0 + G, :],
                            ud[:], ALU.min,
                        )
                    else:
                        nc.vector.tensor_tensor(fd[:], fd[:], ud[:], ALU.min)
                # transpose group planes into L2; ACT evacuates PSUM
                pt = psum.tile([96, GW], BF16, name=f"pt_{g}", tag="pt",
                               bufs=2)
                for k in range(G):
                    nc.tensor.transpose(pt[:, k * 96 : (k + 1) * 96],
                                        fd[:, k, :], ident[:96, :96])
                if EVAC_ON_GP:
                    # window-1 avg-pool == copy; runs on the idle GPSIMD
                    nc.gpsimd.pool(
                        f2[:, g0 : g0 + G, PAD : PAD + 96],
                        pt[:, :].rearrange("p (k w) -> p (k w) 1"),
                        mybir.PoolFunctionType.avg,
                    )
                else:
                    nc.scalar.activation(
                        f2[:, g0 : g0 + G, PAD : PAD + 96],
                        pt[:, :].rearrange("p (k w) -> p k w", k=G),
                        AF.Copy,
                    )

            def emit_h_body(g):
                g0 = g * G
                eng = nc.gpsimd if H_ON_GP[g] else nc.vector
                _edt_range(eng, pool, f2f, fhf, g0 * Lh, (g0 + G) * Lh, S,
                           f"h{g}")
                # transpose back into PSUM (evac'd later by the sqrt)
                ptb = psum.tile([96, GW], BF16, name=f"ptb_{g}", tag="pt",
                                bufs=2)
                for k in range(G):
                    nc.tensor.transpose(
                        ptb[:, k * 96 : (k + 1) * 96],
                        fh[:, g0 + k, PAD : PAD + 96], ident[:96, :96],
                    )
                return ptb

            def emit_h_tail(g, ptb):
                # evac fuses y = sqrt(d2)/theta; w = exp(-y) with free sum(w)
                g0 = g * G
                nc.scalar.activation(
                    y[:, g0 : g0 + G, :],
                    ptb[:, :].rearrange("p (k w) -> p k w", k=G),
                    AF.Sqrt, scale=1.0 / (THETA * THETA),
                )
                nc.scalar.activation(
                    wgt[:, g * GW : (g + 1) * GW],
                    y[:, g0 : g0 + G, :].rearrange("p a b -> p (a b)"),
                    AF.Exp, scale=-1.0, accum_out=accT[:, g : g + 1],
                )

            # ---- EDT emission: D groups (W-pass shipped from host)
            emit_d_group(0)
            emit_d_group(1)
            emit_d_group(2)
            # ---- loss bulk ACT work (emitted per EMIT_ORDER interleave)
            NE = N_E_CHUNKS  # e chunks (fine so ACT can yield to evacs)
            e = pool.tile([96, C, CW], BF16, name="e")
            ef = e.rearrange("h c f -> h (c f)")
            EC = C * CW // NE

            def emit_e(i):
                nc.scalar.activation(ef[:, i * EC : (i + 1) * EC],
                                     Pf[:, i * EC : (i + 1) * EC], AF.Exp)

            e2t = pool.tile([96, CW], BF16, name="e2t")
            e2 = pool.tile([96, C, CW], BF16, name="e2")

            def emit_e2(c):
                if c < N_E2_ACT:
                    nc.scalar.activation(e2[:, c, :], P_[:, c, :], AF.Exp,
                                         scale=2.0)
                elif E2_BY_GROUP:
                    # c encodes (group, class): finer chunks unblock the
                    # per-group sp consumers earlier
                    gg, cc = divmod(c, C)
                    sl = slice(gg * GW, (gg + 1) * GW)
                    nc.vector.tensor_tensor(e2[:, cc, sl], e[:, cc, sl],
                                            e[:, cc, sl], ALU.mult)
                else:
                    nc.vector.tensor_tensor(e2[:, c, :], e[:, c, :],
                                            e[:, c, :], ALU.mult)

            zp = pool.tile([96, 2, CW], BF16, name="zp")
            Z = pool.tile([96, CW], BF16, name="Z")
            sp = pool.tile([96, 2, CW], BF16, name="sp")
            S2 = pool.tile([96, CW], BF16, name="S2")
            lnZ = pool.tile([96, CW], F32, name="lnZ")
            r = pool.tile([96, CW], BF16, name="r")
            ptbs = [None] * NG
            for item in EMIT_ORDER:
                kind, idx = item
                if kind == "h":
                    ptbs[idx] = emit_h_body(idx)
                elif kind == "e":
                    emit_e(idx)
                elif kind == "e2":
                    emit_e2(idx)
                elif kind == "e2t":
                    nc.scalar.activation(e2t[:, :], pt2[:, :], AF.Exp)
                elif kind == "zp":
                    if LADDER_HALVES:
                        for hh in range(2):
                            sl = slice(hh * CW // 2, (hh + 1) * CW // 2)
                            nc.vector.tensor_tensor(
                                zp[:, :, sl], e[:, 0:2, sl], e[:, 2:4, sl],
                                ALU.add)
                            nc.vector.tensor_tensor(
                                Z[:, sl], zp[:, 0, sl], zp[:, 1, sl], ALU.add)
                            nc.scalar.activation(lnZ[:, sl], Z[:, sl], AF.Ln)
                            nc.scalar.activation(r[:, sl], lnZ[:, sl], AF.Exp,
                                                 scale=-1.0)
                    else:
                        nc.vector.tensor_tensor(zp[:], e[:, 0:2, :],
                                                e[:, 2:4, :], ALU.add)
                        nc.vector.tensor_tensor(Z[:], zp[:, 0, :],
                                                zp[:, 1, :], ALU.add)

            # ---- softmax chain (Z/S2 pair-adds emitted via EMIT_ORDER)
            if not LADDER_HALVES:
                nc.scalar.activation(lnZ[:, :], Z[:, :], AF.Ln)
                nc.scalar.activation(r[:, :], lnZ[:, :], AF.Exp, scale=-1.0)
            # per-group tail: t4 = r*(S2*r - e2t) factored (no r2 op);
            # group chains interleave with the H-pass groups on DVE
            m1 = pool.tile([96, CW], BF16, name="m1")

            def emit_tail_slice(g, sl, hh):
                nc.vector.tensor_tensor(sp[:, :, sl], e2[:, 0:2, sl],
                                        e2[:, 2:4, sl], ALU.add)
                nc.vector.tensor_tensor(S2[:, sl], sp[:, 0, sl],
                                        sp[:, 1, sl], ALU.add)
                if WR_FOLD:
                    nc.vector.tensor_tensor(t4[:, sl], wgt[:, sl], r[:, sl],
                                            ALU.mult)
                nc.vector.tensor_tensor(m1[:, sl], S2[:, sl], r[:, sl],
                                        ALU.mult)
                nc.vector.tensor_tensor(m1[:, sl], m1[:, sl], e2t[:, sl],
                                        ALU.subtract)
                if WR_FOLD:
                    nc.vector.tensor_tensor(junk[:, sl], m1[:, sl], t4[:, sl],
                                            ALU.mult)
                else:
                    nc.vector.tensor_tensor(t4[:, sl], m1[:, sl], r[:, sl],
                                            ALU.mult)
                    nc.vector.tensor_tensor(junk[:, sl], t4[:, sl], wgt[:, sl],
                                            ALU.mult)
                if LAST_RED_DVE and g == NG - 1:
                    nc.vector.tensor_reduce(
                        accT[:, NG + g + hh : NG + g + hh + 1],
                        junk[:, sl], op=ALU.add,
                        axis=mybir.AxisListType.X,
                    )
                else:
                    nc.scalar.activation(
                        y[:, g * G : (g + 1) * G, :],
                        junk[:, sl].rearrange("p (a b) -> p a b", b=96),
                        AF.Copy, accum_out=accT[:, NG + g : NG + g + 1],
                    )

            for g in range(NG):
                emit_h_tail(g, ptbs[g])
                nhalf = 2 if (SPLIT_LAST_TAIL and g == NG - 1) else 1
                for hh in range(nhalf):
                    a0 = g * GW + hh * GW // nhalf
                    emit_tail_slice(g, slice(a0, a0 + GW // nhalf), hh)

            nc.sync.dma_start(out_part[:, :], accT[:, :])

    _split_multi_waits(nc)
    return nc


_cache: dict[int, bass.Bass] = {}


def make_in_maps(pred: np.ndarray, target: np.ndarray, S: int,
                 dist: np.ndarray) -> list:
    E = DS + 2 * S
    # W-pass output: squared 1-D W-line distance (<= S^2, exact in bf16);
    # BIG where the line has no boundary voxel
    seed_full = np.where(
        dist < (1 << 20), (dist.astype(np.int64) ** 2).astype(np.float64), BIG
    ).astype(ml_dtypes.bfloat16)                                     # (B,D,H,W)
    pred_bf = pred.astype(ml_dtypes.bfloat16)
    # host gather of the target-class logit, with ln2 folded in
    pt2_full = (
        np.take_along_axis(pred, target[:, None], axis=1)[:, 0] + LN2
    ).astype(ml_dtypes.bfloat16)                                     # (B,D,H,W)
    in_maps = []
    for core in range(N_CORES):
        b, i = divmod(core, 4)
        d0 = i * DS
        dg = np.arange(d0 - S, d0 + DS + S)          # global plane ids, may be OOR
        inr = (dg >= 0) & (dg < D)
        seed = np.full((E, H, 96), BIG, ml_dtypes.bfloat16)
        seed[inr] = seed_full[b][dg[inr]]
        in_maps.append({
            "seed": np.ascontiguousarray(
                seed.transpose(1, 0, 2).reshape(H, E * 96)
            ),
            "predh": np.ascontiguousarray(
                pred_bf[b, :, d0 : d0 + DS].transpose(2, 0, 1, 3)
            ).reshape(H, C * DS * W),
            "predt2": np.ascontiguousarray(
                pt2_full[b, d0 : d0 + DS].transpose(1, 0, 2)
            ).reshape(H, DS * W),
        })
    return in_maps


def kernel(pred: np.ndarray, target: np.ndarray) -> np.ndarray:
    pred = np.ascontiguousarray(pred, np.float32)
    target = np.ascontiguousarray(target, np.int32)
    dist = _wline_dist(target)
    S = _window_for(dist)

    if S not in _cache:
        _cache[S] = build_nc(S)
    nc = _cache[S]

    in_maps = make_in_maps(pred, target, S, dist)
    res = run_bass_kernel_spmd(nc, in_maps, core_ids=list(range(N_CORES)))
    total = sum(float(r["partial"].sum()) for r in res.results)
    n_vox = float(B * D * H * W)
    return np.array(total / n_vox, dtype=np.float32)
